# revision 7
# baseline (speedup 1.0000x reference)
"""RealFormer encoder layer (nn_EncoderLayer) on 8 Trainium2 NeuronCores.

Sharding: each core owns (batch b, query-half sh) — b = core//2, sh = core%2.
All compute for that slice is local; there are no collectives. For sh=1 cores
the token axis is rotated by 512 on the host so every core's query block sits
at token columns 0-511 of its shard (one SPMD program serves all cores); the
host un-rotates the raw output rows afterwards. Attention sums over tokens are
permutation-invariant, so nothing else changes.

Everything on-device works in "transposed activation" space:
  scores^T[t, s], raw^T[t, s], e^T[t, s], att^T[dk, s], attention^T[d, s].
The host only slices / transposes inputs into the layouts the kernel wants and
transposes the per-core outputs back into the reference layout.

Math mapping (layer_ind = l):
  raw = (masked(q@kT * dk^-0.5) + prev) / 2
      = (q*0.0625)@kT + (prev*0.5 + qmask_-inf + kmask_-inf)   [q pre-scaled]
  cor = raw / (1 - 0.5^l);  e = exp(cor)  (no max-subtraction: |cor| < ~10)
  w   = e / max(sum_t e, 1e-30)   (reference: all-masked rows -> 0)
v is augmented with a ones column, so row 64 of each head's att-matmul PSUM
holds sum_t e. All matmuls run as float32r (~1.5e-4 rel err, full PE rate).
"""

import numpy as np

B, S, D, H, F = 4, 1024, 1024, 16, 4096
DK = D // H          # 64
SH = S // 2          # 512 query rows per core
NCORES = 8
P = 128
NEG_INF = np.float32(-np.inf)


def _build(e_scale: float):
    from concourse import bacc
    import concourse.mybir as mybir
    import concourse.tile as tile

    f32 = mybir.dt.float32
    f32r = mybir.dt.float32r
    ACT = mybir.ActivationFunctionType

    nc = bacc.Bacc("TRN2", target_bir_lowering=False, debug=False)

    srcT_d = nc.dram_tensor("srcT", (D, S), f32, kind="ExternalInput").ap()
    prevM_d = nc.dram_tensor("prevM", (H, S, SH), f32, kind="ExternalInput").ap()
    wq_d = nc.dram_tensor("wq", (8, 8, P, P), f32r, kind="ExternalInput").ap()
    wk_d = nc.dram_tensor("wk", (8, 8, P, P), f32r, kind="ExternalInput").ap()
    wv_d = nc.dram_tensor("wv", (D, H * (DK + 1)), f32r, kind="ExternalInput").ap()
    wp_d = nc.dram_tensor("wp", (H, 8, DK, P), f32r, kind="ExternalInput").ap()
    w1_d = nc.dram_tensor("w1", (32, 8, P, P), f32r, kind="ExternalInput").ap()
    w2_d = nc.dram_tensor("w2", (F, D), f32r, kind="ExternalInput").ap()
    g1c_d = nc.dram_tensor("g1c", (P, 8), f32, kind="ExternalInput").ap()
    g2c_d = nc.dram_tensor("g2c", (P, 8), f32, kind="ExternalInput").ap()
    b1c_d = nc.dram_tensor("b1c", (P, 32), f32, kind="ExternalInput").ap()
    b2c_d = nc.dram_tensor("b2c", (P, 8), f32, kind="ExternalInput").ap()
    ones_d = nc.dram_tensor("ones", (P, H), f32r, kind="ExternalInput").ap()

    rawT_d = nc.dram_tensor("rawT", (H, S, SH), f32, kind="ExternalOutput").ap()
    outT_d = nc.dram_tensor("outT", (D, SH), f32, kind="ExternalOutput").ap()

    with tile.TileContext(nc, pool_alloc_mode="queue") as tc:
        with tc.tile_pool(name="pers", bufs=1) as pers:
            g1c = pers.tile([P, 8], f32)
            nc.sync.dma_start(g1c, g1c_d)
            g2c = pers.tile([P, 8], f32)
            nc.sync.dma_start(g2c, g2c_d)
            b1c = pers.tile([P, 32], f32)
            nc.sync.dma_start(b1c, b1c_d)
            b2c = pers.tile([P, 8], f32)
            nc.sync.dma_start(b2c, b2c_d)
            ones1 = pers.tile([P, 1], f32r)
            nc.sync.dma_start(ones1, ones_d[:, 0:1])
            attnT = pers.tile([P, 8, 512], f32)

            with tc.tile_pool(name="pQKVo", bufs=1) as pqv:
                qT = pqv.tile([P, 8, 512], f32r)     # [2-head dk, pair, s]
                kT = pqv.tile([P, 8, S], f32r)       # [2-head dk, pair, t]
                v_aug = pqv.tile([P, 8, 8 * 130], f32r)  # [t, tj, pair*130]

                # ---------- stage A: rmsnorm1 -> xn ----------
                with tc.tile_pool(name="pXN", bufs=1) as pxn:
                    xn = pxn.tile([P, 8, S], f32r)
                    with tc.tile_pool(name="pA", bufs=1) as pa, \
                         tc.tile_pool(name="sqp", bufs=3) as sqp, \
                         tc.tile_pool(name="psStat", bufs=1, space="PSUM") as pst:
                        srcT = pa.tile([P, 8, S], f32)
                        nc.sync.dma_start(
                            srcT, srcT_d.rearrange("(dt p) t -> p dt t", p=P))
                        ps_ss = [pst.tile([1, 512], f32, tag=f"ps_ss{i}",
                                          name=f"ps_ss{i}")
                                 for i in range(2)]
                        for dt in range(8):
                            sq = sqp.tile([P, S], f32r, tag="sq")
                            nc.scalar.activation(out=sq, in_=srcT[:, dt, :],
                                                 func=ACT.Square)
                            for nh in range(2):
                                nc.tensor.matmul(
                                    ps_ss[nh], ones1,
                                    sq[:, nh * 512:(nh + 1) * 512],
                                    start=(dt == 0), stop=(dt == 7))
                        rinv1 = pa.tile([1, S], f32)
                        for nh in range(2):
                            nc.scalar.activation(
                                out=rinv1[:, nh * 512:(nh + 1) * 512],
                                in_=ps_ss[nh], func=ACT.Sqrt, scale=1.0 / D)
                        nc.vector.reciprocal(rinv1, rinv1)
                        rinv1_b = pa.tile([P, S], f32)
                        nc.gpsimd.partition_broadcast(rinv1_b, rinv1)
                        for dt in range(8):
                            tmp = sqp.tile([P, S], f32, tag="xtmp")
                            nc.vector.tensor_mul(tmp, srcT[:, dt, :], rinv1_b)
                            nc.vector.tensor_scalar_mul(
                                xn[:, dt, :], tmp, g1c[:, dt:dt + 1])

                    # ---------- stage B: q, k, v ----------
                    with tc.tile_pool(name="wqk", bufs=4) as wqk, \
                         tc.tile_pool(name="psQK", bufs=3, space="PSUM") as psqk:
                        for pr in range(8):
                            ps_q = psqk.tile([P, 512], f32, tag="mm")
                            for dt in range(8):
                                wt = wqk.tile([P, P], f32r, tag="wqt")
                                nc.sync.dma_start(wt, wq_d[pr, dt])
                                nc.tensor.matmul(ps_q, wt, xn[:, dt, 0:512],
                                                 start=(dt == 0), stop=(dt == 7))
                            nc.scalar.mul(qT[:, pr, :], ps_q, 0.0625)
                        for pr in range(8):
                            for nh in range(2):
                                ps_k = psqk.tile([P, 512], f32, tag="mm")
                                for dt in range(8):
                                    wt = wqk.tile([P, P], f32r, tag="wkt")
                                    nc.sync.dma_start(wt, wk_d[pr, dt])
                                    nc.tensor.matmul(
                                        ps_k, wt,
                                        xn[:, dt, nh * 512:(nh + 1) * 512],
                                        start=(dt == 0), stop=(dt == 7))
                                nc.scalar.copy(
                                    kT[:, pr, nh * 512:(nh + 1) * 512], ps_k)
                    with tc.tile_pool(name="pWV", bufs=1) as pwv, \
                         tc.tile_pool(name="psV", bufs=1, space="PSUM") as psv:
                        wv_sb = pwv.tile([P, 8, H * (DK + 1)], f32r)
                        nc.sync.dma_start(
                            wv_sb, wv_d.rearrange("(dt p) m -> p dt m", p=P))
                        for tj in range(8):
                            ps_v = [psv.tile([P, 130], f32, tag=f"ps_v{pr}",
                                             name=f"ps_v{pr}")
                                    for pr in range(8)]
                            for dt in range(8):
                                for pr in range(8):
                                    nc.tensor.matmul(
                                        ps_v[pr],
                                        xn[:, dt, tj * P:(tj + 1) * P],
                                        wv_sb[:, dt, pr * 130:(pr + 1) * 130],
                                        start=(dt == 0), stop=(dt == 7))
                            for pr in range(8):
                                nc.vector.tensor_copy(
                                    v_aug[:, tj, pr * 130:(pr + 1) * 130],
                                    ps_v[pr])
                            nc.sync.dma_start(v_aug[:, tj, 64::65], ones_d)

                # ---------- stage C: attention ----------
                with tc.tile_pool(name="pAtt", bufs=1) as patt:
                    att_all = patt.tile([DK, H, 512], f32r)  # partitions 0-63
                    with tc.tile_pool(name="ebufs", bufs=2) as ebp, \
                         tc.tile_pool(name="attio", bufs=3) as aio, \
                         tc.tile_pool(name="denp", bufs=2) as dnp, \
                         tc.tile_pool(name="psS", bufs=3, space="PSUM") as pss, \
                         tc.tile_pool(name="psT", bufs=2, space="PSUM") as pstt:
                        for h in range(H):
                            pr, hh = h // 2, h % 2
                            eb = ebp.tile([P, 8, 512], f32r, tag="ebuf")
                            for tj in range(8):
                                ps_s = pss.tile([P, 512], f32, tag="ps_s")
                                nc.tensor.matmul(
                                    ps_s,
                                    kT[hh * 64:(hh + 1) * 64, pr,
                                       tj * P:(tj + 1) * P],
                                    qT[hh * 64:(hh + 1) * 64, pr, :],
                                    start=True, stop=True,
                                    tile_position=(hh * 64, 0))
                                pm = aio.tile([P, 512], f32, tag="pm")
                                nc.sync.dma_start(
                                    pm, prevM_d[h, tj * P:(tj + 1) * P, :])
                                raw_t = aio.tile([P, 512], f32, tag="raw")
                                nc.vector.tensor_add(raw_t, ps_s, pm)
                                nc.sync.dma_start(
                                    rawT_d[h, tj * P:(tj + 1) * P, :], raw_t)
                                nc.scalar.activation(
                                    out=eb[:, tj, :], in_=raw_t,
                                    func=ACT.Exp, scale=e_scale)
                            ps_att = pstt.tile([DK + 1, 512], f32, tag="ps_att")
                            for tj in range(8):
                                nc.tensor.matmul(
                                    ps_att, v_aug[:, tj, h * 65:(h + 1) * 65],
                                    eb[:, tj, :],
                                    start=(tj == 0), stop=(tj == 7))
                            den_hi = dnp.tile([DK + 1, 512], f32, tag="den_hi")
                            nc.vector.tensor_scalar_max(
                                den_hi[64:65, :], ps_att[64:65, :], 1e-30)
                            nc.vector.reciprocal(
                                den_hi[64:65, :], den_hi[64:65, :])
                            den_lo = dnp.tile([1, 512], f32, tag="den_lo")
                            nc.sync.dma_start(den_lo, den_hi[64:65, :])
                            den_b = dnp.tile([DK, 512], f32, tag="den_b")
                            nc.gpsimd.partition_broadcast(den_b, den_lo)
                            nc.vector.tensor_mul(att_all[:, h, :],
                                                 ps_att[0:64, :], den_b)

                    # ---------- stage D: proj + residual ----------
                    with tc.tile_pool(name="wpp", bufs=4) as wpp, \
                         tc.tile_pool(name="resp", bufs=3) as resp, \
                         tc.tile_pool(name="psP", bufs=2, space="PSUM") as psp:
                        for dch in range(8):
                            ps_p = psp.tile([P, 512], f32, tag="ps_p")
                            for h in range(H):
                                wpt = wpp.tile([DK, P], f32r, tag="wpt")
                                nc.sync.dma_start(wpt, wp_d[h, dch])
                                nc.tensor.matmul(ps_p, wpt, att_all[:, h, :],
                                                 start=(h == 0), stop=(h == 15))
                            rs = resp.tile([P, 512], f32, tag="rs")
                            nc.sync.dma_start(
                                rs, srcT_d[dch * P:(dch + 1) * P, 0:512])
                            nc.vector.tensor_add(attnT[:, dch, :], ps_p, rs)

            # ---------- stage E: rmsnorm2 ----------
            with tc.tile_pool(name="pFFN", bufs=1) as pffn:
                ln2 = pffn.tile([P, 8, 512], f32r)
                with tc.tile_pool(name="n2", bufs=3) as n2p, \
                     tc.tile_pool(name="psN", bufs=1, space="PSUM") as psn:
                    ps_ss2 = psn.tile([1, 512], f32, tag="ps_ss2")
                    for dch in range(8):
                        sq2 = n2p.tile([P, 512], f32r, tag="sq2")
                        nc.scalar.activation(out=sq2, in_=attnT[:, dch, :],
                                             func=ACT.Square)
                        nc.tensor.matmul(ps_ss2, ones1, sq2,
                                         start=(dch == 0), stop=(dch == 7))
                    rinv2 = n2p.tile([1, 512], f32, tag="rinv2")
                    nc.scalar.activation(out=rinv2, in_=ps_ss2, func=ACT.Sqrt,
                                         scale=1.0 / D)
                    nc.vector.reciprocal(rinv2, rinv2)
                    rinv2_b = n2p.tile([P, 512], f32, tag="rinv2b")
                    nc.gpsimd.partition_broadcast(rinv2_b, rinv2)
                    for dch in range(8):
                        t2 = n2p.tile([P, 512], f32, tag="t2")
                        nc.vector.tensor_mul(t2, attnT[:, dch, :], rinv2_b)
                        nc.vector.tensor_scalar_mul(ln2[:, dch, :], t2,
                                                    g2c[:, dch:dch + 1])

                # ---------- stage F: FFN1 ----------
                h_sb = pffn.tile([P, 32, 512], f32r)
                with tc.tile_pool(name="w1p", bufs=3) as w1p, \
                     tc.tile_pool(name="psF", bufs=3, space="PSUM") as psf:
                    for fc in range(32):
                        w1t = w1p.tile([P, 8, P], f32r, tag="w1t")
                        nc.sync.dma_start(
                            w1t, w1_d[fc].rearrange("j p f -> p j f"))
                        ps_f = psf.tile([P, 512], f32, tag="ps_f")
                        for dt in range(8):
                            nc.tensor.matmul(ps_f, w1t[:, dt, :], ln2[:, dt, :],
                                             start=(dt == 0), stop=(dt == 7))
                        nc.scalar.activation(out=h_sb[:, fc, :], in_=ps_f,
                                             func=ACT.Relu,
                                             bias=b1c[:, fc:fc + 1], scale=1.0)

                # ---------- stage G: FFN2 + bias + residual ----------
                with tc.tile_pool(name="w2p", bufs=3) as w2p, \
                     tc.tile_pool(name="outp", bufs=3) as outp, \
                     tc.tile_pool(name="psO", bufs=1, space="PSUM") as pso:
                    for ph in range(2):
                        ps_o = [pso.tile([P, 512], f32, tag=f"ps_o{i}",
                                         name=f"ps_o{i}")
                                for i in range(4)]
                        for ft in range(32):
                            w2t = w2p.tile([P, 512], f32r, tag="w2t")
                            nc.sync.dma_start(
                                w2t, w2_d[ft * P:(ft + 1) * P,
                                          ph * 512:(ph + 1) * 512])
                            for i in range(4):
                                nc.tensor.matmul(
                                    ps_o[i], w2t[:, i * P:(i + 1) * P],
                                    h_sb[:, ft, :],
                                    start=(ft == 0), stop=(ft == 31))
                        for i in range(4):
                            dch = ph * 4 + i
                            ot = outp.tile([P, 512], f32, tag="ot")
                            nc.scalar.activation(out=ot, in_=ps_o[i],
                                                 func=ACT.Identity,
                                                 bias=b2c[:, dch:dch + 1])
                            ot2 = outp.tile([P, 512], f32, tag="ot2")
                            nc.vector.tensor_add(ot2, ot, attnT[:, dch, :])
                            nc.sync.dma_start(
                                outT_d[dch * P:(dch + 1) * P, :], ot2)

    nc.compile()
    return nc


_NC_CACHE = {}
_LAST_RESULTS = None


def _get_nc(e_scale):
    if e_scale not in _NC_CACHE:
        _NC_CACHE[e_scale] = _build(e_scale)
    return _NC_CACHE[e_scale]


def _prep_in_maps(src, src_padding_mask, prev, layer_ind, Wq, Wk, Wv, Wproj,
                  gamma1, gamma2, W1, b1, W2, b2):
    src = np.asarray(src, np.float32)
    mask = np.asarray(src_padding_mask).astype(bool)
    prev = np.asarray(prev, np.float32)
    Wq = np.asarray(Wq, np.float32)
    Wk = np.asarray(Wk, np.float32)
    Wv = np.asarray(Wv, np.float32)
    Wproj = np.asarray(Wproj, np.float32)
    gamma1 = np.asarray(gamma1, np.float32)
    gamma2 = np.asarray(gamma2, np.float32)
    W1 = np.asarray(W1, np.float32)
    b1 = np.asarray(b1, np.float32)
    W2 = np.asarray(W2, np.float32)
    b2 = np.asarray(b2, np.float32)
    l_ind = int(np.asarray(layer_ind))
    e_scale = float(1.0 / (1.0 - 0.5 ** l_ind))

    # shared (core-independent) input prep
    wq_all = Wq.transpose(1, 0, 2).reshape(D, H * DK)   # [d, h*dk]
    wk_all = Wk.transpose(1, 0, 2).reshape(D, H * DK)
    wq_r = np.ascontiguousarray(
        wq_all.reshape(8, P, 8, P).transpose(2, 0, 1, 3))  # [pr, dt, p, f]
    wk_r = np.ascontiguousarray(
        wk_all.reshape(8, P, 8, P).transpose(2, 0, 1, 3))
    wv_aug = np.zeros((D, H * (DK + 1)), np.float32)
    wv_aug.reshape(D, H, DK + 1)[:, :, :DK] = Wv.transpose(1, 0, 2)
    wp_r = np.ascontiguousarray(
        Wproj.reshape(H, DK, 8, P).transpose(0, 2, 1, 3))  # [h, dch, dk, f]
    w1_r = np.ascontiguousarray(
        W1.reshape(8, P, 32, P).transpose(2, 0, 1, 3))     # [fc, dt, p, f]
    g1c = np.ascontiguousarray(gamma1.reshape(8, P).T)
    g2c = np.ascontiguousarray(gamma2.reshape(8, P).T)
    b1c = np.ascontiguousarray(b1.reshape(32, P).T)
    b2c = np.ascontiguousarray(b2.reshape(8, P).T)
    ones = np.ones((P, H), np.float32)

    in_maps = []
    for core in range(NCORES):
        b, sh = core // 2, core % 2
        soff = sh * SH
        srcT = src[b].T                                     # [D, S]
        kvadd = np.where(mask[b], NEG_INF, 0.0).astype(np.float32)   # over t
        qadd = np.where(mask[b, soff:soff + SH], NEG_INF,
                        0.0).astype(np.float32)                      # over s
        prevM = prev[:, b, soff:soff + SH, :].transpose(0, 2, 1) * 0.5
        prevM = prevM + kvadd[None, :, None] + qadd[None, None, :]
        if sh == 1:
            # rotate token axis so this core's query block is at t in [0, 512)
            roll = np.r_[SH:S, 0:SH]
            srcT = srcT[:, roll]
            prevM = prevM[:, roll, :]
        in_maps.append({
            "srcT": np.ascontiguousarray(srcT),
            "prevM": np.ascontiguousarray(prevM.astype(np.float32)),
            "wq": wq_r, "wk": wk_r, "wv": wv_aug, "wp": wp_r,
            "w1": w1_r, "w2": W2, "g1c": g1c, "g2c": g2c,
            "b1c": b1c, "b2c": b2c, "ones": ones,
        })

    return in_maps, e_scale


def kernel(src, src_padding_mask, prev, layer_ind, Wq, Wk, Wv, Wproj,
           gamma1, gamma2, W1, b1, W2, b2, **kw):
    from concourse.bass_utils import run_bass_kernel_spmd

    in_maps, e_scale = _prep_in_maps(
        src, src_padding_mask, prev, layer_ind, Wq, Wk, Wv, Wproj,
        gamma1, gamma2, W1, b1, W2, b2)
    nc = _get_nc(e_scale)
    res = run_bass_kernel_spmd(nc, in_maps, core_ids=list(range(NCORES)))
    global _LAST_RESULTS
    _LAST_RESULTS = res

    out = np.empty((B, S, D), np.float32)
    raw = np.empty((H, B, S, S), np.float32)
    for core in range(NCORES):
        b, sh = core // 2, core % 2
        soff = sh * SH
        r = res.results[core]
        out[b, soff:soff + SH, :] = r["outT"].T
        rawT = r["rawT"]                       # [H, t(maybe rolled), s]
        if sh == 1:
            rawT = rawT[:, np.r_[SH:S, 0:SH], :]
        raw[:, b, soff:soff + SH, :] = rawT.transpose(0, 2, 1)
    return out, raw
